# revision 13
# baseline (speedup 1.0000x reference)
"""AttentionRNN Trainium2 kernel: 8-core data-parallel over batch.

Per-core layout (b=16 samples/core), everything feature-on-partition:
  - recurrent state hT/cT: [128, 8, 16]  (h-chunk on partitions, (k,b) free)
  - gate preacts psum:      [128, 40, 16] (5 gates x 8 h-chunks on j-tiles)
  - weights pre-transposed/gate-permuted on host into lhsT tile layouts (bf16)
  - pi = x@Wi.T + all biases precomputed once into DRAM scratch, bf16
  - attention: va (a-on-partition), logits via PE dot with Wp, softmax on
    one partition, att transposed to [64n,16b] via tiny SBUF->SBUF DMA,
    ctx via per-(b,vchunk) matmuls with visual as stationary operand.
Gate order permuted to (i, f, o, hw, m) so sigmoid is one contiguous slab.
"""

import os
import sys
import numpy as np

sys.path.insert(0, "/opt/trn_rl_repo")

B, T, D_IN, H, V, A, N = 128, 32, 1024, 1024, 2048, 512, 64
NC_ = 8
BB = B // NC_          # 16 samples per core
P = 128
KH = H // P            # 8  h-chunks
KV = V // P            # 16 v-chunks
AT = A // P            # 4  a-tiles
JT = 5 * H // P        # 40 gate j-tiles
JI = 6 * H // P        # 48 pi j-tiles (40 gates + 8 highway)
NTB = T * BB           # 512 (t,b) columns

# gate permutation: orig rows blocks (i,f,m,o,hw) -> new order (i,f,o,hw,m)
GPERM = [0, 1, 3, 4, 2]


def _orig_row_base(jt):
    """DRAM row base in the original 5H (or 6H) weight for new j-tile jt."""
    if jt < JT:
        g_new, hc = jt // KH, jt % KH
        return GPERM[g_new] * H + hc * P
    # highway block (pi only): orig rows 5H..6H
    return 5 * H + (jt - JT) * P


def _prep_core_inputs(inputs, core):
    f32, bf16 = np.float32, np.bfloat16 if hasattr(np, "bfloat16") else None
    import ml_dtypes
    bf16 = ml_dtypes.bfloat16

    sl = slice(core * BB, (core + 1) * BB)
    x = np.asarray(inputs["x"], f32)[sl]          # [16,32,1024]
    vis = np.asarray(inputs["visual"], f32)[sl]   # [16,64,2048]
    ls = np.asarray(inputs["ls_rois"])[sl].astype(np.int64)
    sq = np.asarray(inputs["seq_lens"])[sl].astype(np.int64)

    Wi = np.asarray(inputs["Wi"], f32); bi = np.asarray(inputs["bi"], f32)
    Ws = np.asarray(inputs["Ws"], f32); bs = np.asarray(inputs["bs"], f32)
    Wa = np.asarray(inputs["Wa"], f32); ba = np.asarray(inputs["ba"], f32)
    Wv = np.asarray(inputs["Wv"], f32); bv = np.asarray(inputs["bv"], f32)
    Wh = np.asarray(inputs["Wh"], f32); bh = np.asarray(inputs["bh"], f32)
    Wp = np.asarray(inputs["Wp"], f32)
    W0h = np.asarray(inputs["W0h"], f32); b0h = np.asarray(inputs["b0h"], f32)
    W0c = np.asarray(inputs["W0c"], f32); b0c = np.asarray(inputs["b0c"], f32)
    Wo = np.asarray(inputs["Wo"], f32)

    m = {}
    # xT [128, 8, 512]: xT[p,k,t*16+b] = x[b,t,k*128+p]
    xt = x.transpose(2, 1, 0).reshape(KH, P, T * BB)        # [k*p? ] careful
    # x.transpose(2,1,0): [1024 d, 32 t, 16 b] -> reshape [8, 128, 32*16]
    xt = x.transpose(2, 1, 0).reshape(KH, P, T, BB).reshape(KH, P, NTB)
    m["xT"] = np.ascontiguousarray(xt.transpose(1, 0, 2)).astype(bf16)  # [128,8,512]

    # wiT [48, 128, 8, 128]: wiT[j,p,k,q] = Wi[orig_row(j)+q, k*128+p]
    wiT = np.empty((JI, P, KH, P), f32)
    for j in range(JI):
        r0 = _orig_row_base(j)
        blk = Wi[r0:r0 + P, :].reshape(P, KH, P)            # [q, k, p]
        wiT[j] = blk.transpose(2, 1, 0)                     # [p, k, q]
    m["wiT"] = wiT.astype(bf16)

    # wsT [128, 40, 8, 128]: wsT[p,j,k,q] = Ws[orig_row(j)+q, k*128+p]
    wsT = np.empty((JT, P, KH, P), f32)
    for j in range(JT):
        r0 = _orig_row_base(j)
        wsT[j] = Ws[r0:r0 + P, :].reshape(P, KH, P).transpose(2, 1, 0)
    m["wsT"] = np.ascontiguousarray(wsT.transpose(1, 0, 2, 3)).astype(bf16)  # [128,40,8,128]

    # waT [40, 128, 16, 128]: waT[j,p,v,q] = Wa[orig_row(j)+q, v*128+p]
    waT = np.empty((JT, P, KV, P), f32)
    for j in range(JT):
        r0 = _orig_row_base(j)
        waT[j] = Wa[r0:r0 + P, :].reshape(P, KV, P).transpose(2, 1, 0)
    m["waT"] = waT.astype(bf16)

    # whT [128, 4, 8, 128]: whT[p,a,k,q] = Wh[a*128+q, k*128+p]
    whT = Wh.reshape(AT, P, KH, P).transpose(3, 0, 2, 1)
    m["whT"] = np.ascontiguousarray(whT).astype(bf16)

    # w0T [16, 128, 16, 128]: j2<8 -> W0h, j2>=8 -> W0c; w0T[j2,p,k,q]=W[j2%8*128+q, k*128+p]
    w0 = np.concatenate([W0h.reshape(KH, P, KV, P), W0c.reshape(KH, P, KV, P)], 0)
    m["w0T"] = np.ascontiguousarray(w0.transpose(0, 3, 2, 1)).astype(bf16)

    # wvT [4, 128, 16, 128]: wvT[a,p,v,q] = Wv[a*128+q, v*128+p]
    m["wvT"] = np.ascontiguousarray(
        Wv.reshape(AT, P, KV, P).transpose(0, 3, 2, 1)).astype(bf16)

    # visn [128, 8, 2048] bf16 packed pairs: visn[b2*64+n, bp, v] = visual[2*bp+b2, n, v]
    vp = vis.reshape(8, 2, N, V).transpose(1, 2, 0, 3)       # [b2, n, bp, v]
    m["visn"] = np.ascontiguousarray(vp.reshape(P, 8, V)).astype(bf16)

    # visT [128, 16, 16, 64]: visT[p,vc,b,n] = visual[b,n,vc*128+p]
    m["visT"] = np.ascontiguousarray(
        vis.reshape(BB, N, KV, P).transpose(3, 2, 0, 1)).astype(bf16)

    # wpc [128, 4], woc [128, 8]
    m["wpc"] = np.ascontiguousarray(Wp[0].reshape(AT, P).T).astype(bf16)
    m["woc"] = np.ascontiguousarray(Wo[0].reshape(KH, P).T).astype(bf16)

    # wroisT [128, 16] bf16 block-diag: col 2*bp+b2, row b2*64+n
    wr = (np.arange(N)[:, None] < ls[None, :]) / ls[None, :].astype(f32)  # [n, b]
    wz = np.zeros((2, N, BB), f32)
    for b in range(BB):
        wz[b % 2, :, b] = wr[:, b]
    m["wroisT"] = wz.reshape(P, BB).astype(bf16)

    # btot [128, 48] f32: folded bias for pi phase (bi + bs + ba on gate tiles)
    btot = np.empty((P, JI), f32)
    for j in range(JI):
        r0 = _orig_row_base(j)
        b = bi[r0:r0 + P].copy()
        if j < JT:
            b += bs[r0:r0 + P] + ba[r0:r0 + P]
        btot[:, j] = b
    m["btot"] = btot

    # bvh [128, 4]: bv + bh per a-tile
    m["bvh"] = np.ascontiguousarray((bv + bh).reshape(AT, P).T, f32)

    # b0 [128, 16]
    m["b0"] = np.ascontiguousarray(
        np.concatenate([b0h.reshape(KH, P), b0c.reshape(KH, P)], 0).T, f32)

    # rmask [1, 1024] f32, free order (b2, n, bp): idx = b2*512 + n*8 + bp
    rm = np.where(np.arange(N)[:, None] < ls[None, :], 0.0, -1e9).astype(f32)  # [n,b]
    rm2 = np.empty((2, N, 8), f32)
    for b in range(BB):
        rm2[b % 2, :, b // 2] = rm[:, b]
    m["rmask"] = rm2.reshape(1, N * BB)

    # vmask [128, 512] bf16 over (t,b), replicated on partitions
    vm = (np.arange(T)[:, None] < sq[None, :]).astype(f32)   # [32,16]
    m["vmask"] = np.broadcast_to(vm.reshape(1, NTB), (P, NTB)).astype(bf16)

    return m


_CACHE = {}


def _build(bo_val, do_compile=True):
    from contextlib import ExitStack
    import concourse.bacc as bacc
    import concourse.tile as tile
    from concourse import mybir

    nc = bacc.Bacc("TRN2", target_bir_lowering=False, debug=False)
    bf16, f32 = mybir.dt.bfloat16, mybir.dt.float32
    AF = mybir.ActivationFunctionType

    dt_ = {}
    def din(name, shape, dt=bf16):
        dt_[name] = nc.dram_tensor(name, shape, dt, kind="ExternalInput").ap()

    din("xT", [P, KH, NTB]); din("wiT", [JI, P, KH, P])
    din("wsT", [P, JT, KH, P]); din("waT", [JT, P, KV, P])
    din("whT", [P, AT, KH, P]); din("w0T", [2 * KH, P, KV, P])
    din("wvT", [AT, P, KV, P]); din("visn", [P, KH, V])
    din("visT", [P, KV, BB, N])
    din("wpc", [P, AT]); din("woc", [P, KH]); din("wroisT", [P, BB])
    din("btot", [P, JI], f32); din("bvh", [P, AT], f32); din("b0", [P, 2 * KH], f32)
    din("rmask", [1, N * BB], f32); din("vmask", [P, NTB])

    y_dram = nc.dram_tensor("y", [1, NTB], f32, kind="ExternalOutput").ap()

    with ExitStack() as stk:
        tc = stk.enter_context(tile.TileContext(nc))
        const = stk.enter_context(tc.tile_pool(name="const", bufs=1))
        dramp = stk.enter_context(tc.tile_pool(name="dscr", bufs=1, space="DRAM"))
        piT_dram = dramp.tile([T, P, JI, BB], bf16)

        # ---- resident consts ----
        wsT = const.tile([P, JT, KH, P], bf16)
        nc.sync.dma_start(wsT[:], dt_["wsT"][:])
        whT = const.tile([P, AT, KH, P], bf16)
        nc.sync.dma_start(whT[:], dt_["whT"][:])
        visn = const.tile([P, KH, V], bf16)
        nc.sync.dma_start(visn[:], dt_["visn"][:])
        wpc = const.tile([P, AT], bf16)
        nc.sync.dma_start(wpc[:], dt_["wpc"][:])
        woc = const.tile([P, KH], bf16)
        nc.sync.dma_start(woc[:], dt_["woc"][:])
        wroisT = const.tile([P, BB], bf16)
        nc.sync.dma_start(wroisT[:], dt_["wroisT"][:])
        btot = const.tile([P, JI], f32)
        nc.sync.dma_start(btot[:], dt_["btot"][:])
        bvh = const.tile([P, AT], f32)
        nc.sync.dma_start(bvh[:], dt_["bvh"][:])
        b0 = const.tile([P, 2 * KH], f32)
        nc.sync.dma_start(b0[:], dt_["b0"][:])
        rmask = const.tile([1, N * BB], f32)
        nc.sync.dma_start(rmask[:], dt_["rmask"][:])
        vmask = const.tile([P, NTB], bf16)
        nc.sync.dma_start(vmask[:], dt_["vmask"][:])
        attT2z = const.tile([P, BB], bf16)
        nc.vector.memset(attT2z[:], 0.0)
        va_sb = const.tile([P, AT, N * BB], bf16)   # relu input, a-on-partition
        y_sb = const.tile([1, NTB], f32)

        # state (ping-pong)
        hT = [const.tile([P, KH, BB], f32, tag=f"hT{i}", name=f"hT{i}") for i in range(2)]
        cT = [const.tile([P, KH, BB], f32, tag=f"cT{i}", name=f"cT{i}") for i in range(2)]
        hTb = [const.tile([P, KH, BB], bf16, tag=f"hTb{i}", name=f"hTb{i}") for i in range(2)]

        # ================= precompute =================
        with tc.tile_pool(name="pre_sbuf", bufs=3) as pp, \
             tc.tile_pool(name="pre_big", bufs=1) as pb, \
             tc.tile_pool(name="pre_psum", bufs=2, space="PSUM") as qq:

            # ---- mean visual (as matmul with wrois), into mvT bf16 [128, 16v,16b] ----
            ps_mv = qq.tile([P, KV * BB], f32, tag="psA")
            for bp in range(KH):
                for vc in range(KV):
                    nc.tensor.matmul(
                        ps_mv[:, vc * BB + 2 * bp:vc * BB + 2 * bp + 2],
                        visn[:, bp, vc * P:(vc + 1) * P],
                        wroisT[:, 2 * bp:2 * bp + 2], start=True, stop=True)
            mvT = pb.tile([P, KV * BB], bf16)
            nc.vector.tensor_copy(mvT[:], ps_mv[:])

            # ---- h0 / c0 ----
            for half in range(2):
                ps_h0 = qq.tile([P, KH, BB], f32, tag="psA")
                for j in range(KH):
                    j2 = half * KH + j
                    w0t = pp.tile([P, KV, P], bf16, tag="w0s")
                    nc.sync.dma_start(w0t[:], dt_["w0T"][j2])
                    for k in range(KV):
                        nc.tensor.matmul(
                            ps_h0[:, j, :], w0t[:, k, :],
                            mvT[:, k * BB:(k + 1) * BB],
                            start=(k == 0), stop=(k == KV - 1))
                dst = hT[0] if half == 0 else cT[0]
                for j in range(KH):
                    j2 = half * KH + j
                    nc.scalar.activation(dst[:, j, :], ps_h0[:, j, :],
                                         AF.Identity, bias=b0[:, j2:j2 + 1])
            nc.vector.tensor_copy(hTb[0][:], hT[0][:])

            # ---- va = visual @ Wv.T + (bv+bh), free order (b2, n, bp) ----
            for a in range(AT):
                wvt = pp.tile([P, KV, P], bf16, tag="wvs")
                nc.sync.dma_start(wvt[:], dt_["wvT"][a])
                ps_va = qq.tile([P, BB, N], f32, tag="psA")
                for vc in range(KV):
                    vst = pp.tile([P, BB, N], bf16, tag="vst")
                    nc.sync.dma_start(vst[:], dt_["visT"][:, vc])
                    for hh in range(2):
                        nc.tensor.matmul(
                            ps_va[:, hh * KH:(hh + 1) * KH, :], wvt[:, vc, :],
                            vst[:, hh * KH:(hh + 1) * KH, :],
                            start=(vc == 0), stop=(vc == KV - 1))
                # write va col = b2*512 + n*8 + bp from psum (b, n); 2 ACTs by parity
                for b2 in range(2):
                    in_ap = ps_va[:, b2::2, :]
                    out_ap = va_sb[:, a, b2 * 512:(b2 + 1) * 512].rearrange(
                        "p (n bp) -> p bp n", n=N)
                    nc.scalar.activation(out_ap, in_ap, AF.Identity,
                                         bias=bvh[:, a:a + 1])

            # ---- pi precompute into DRAM scratch ----
            xT_s = pb.tile([P, KH, NTB], bf16)
            nc.sync.dma_start(xT_s[:], dt_["xT"][:])
            for j in range(JI):
                wit = pp.tile([P, KH, P], bf16, tag="wis")
                nc.sync.dma_start(wit[:], dt_["wiT"][j])
                ps_pi = qq.tile([P, NTB], f32, tag="psB")
                for k in range(KH):
                    nc.tensor.matmul(ps_pi[:], wit[:, k, :], xT_s[:, k, :],
                                     start=(k == 0), stop=(k == KH - 1))
                pit = pp.tile([P, NTB], bf16, tag="pit")
                nc.scalar.activation(pit[:], ps_pi[:], AF.Identity,
                                     bias=btot[:, j:j + 1])
                # scatter to [T, P, j, BB]
                dst = piT_dram[:, :, j, :].transpose([1, 0, 2])  # iter (p, t, b)
                nc.sync.dma_start(dst, pit[:].rearrange("p (t b) -> p t b", t=T))

        # ================= scan =================
        with tc.tile_pool(name="wa_pool", bufs=3) as wap, \
             tc.tile_pool(name="work", bufs=2) as wk, \
             tc.tile_pool(name="ps_pre", bufs=1, space="PSUM") as pspre, \
             tc.tile_pool(name="ps_small", bufs=1, space="PSUM") as pssm, \
             tc.tile_pool(name="ps_lg", bufs=1, space="PSUM") as pslg:

            for t in range(T):
                cur, nxt = t % 2, (t + 1) % 2
                # -- he --
                ps_he = pssm.tile([P, AT * BB], f32, tag="he")
                for a in range(AT):
                    for k in range(KH):
                        nc.tensor.matmul(ps_he[:, a * BB:(a + 1) * BB],
                                         whT[:, a, k, :], hTb[cur][:, k, :],
                                         start=(k == 0), stop=(k == KH - 1))
                # -- ha = relu(va + he), [128, (n,b)] per a; bf16 out --
                ha = wk.tile([P, AT, N * BB], bf16, tag="ha", bufs=1)
                for a in range(AT):
                    tmp = wk.tile([P, N * BB], f32, tag="hatmp")
                    he_b = ps_he[:, a * BB:(a + 1) * BB].rearrange(
                        "p (bp b2) -> p b2 bp", b2=2).unsqueeze(2).broadcast_to(
                        [P, 2, N, KH])
                    nc.vector.tensor_add(
                        tmp[:].rearrange("p (b2 n bp) -> p b2 n bp", b2=2, n=N),
                        va_sb[:, a, :].rearrange("p (b2 n bp) -> p b2 n bp", b2=2, n=N),
                        he_b)
                    nc.scalar.activation(ha[:, a, :], tmp[:], AF.Relu)
                # -- logits --
                ps_lg = pslg.tile([1, N * BB], f32, tag="lg")
                for hh in range(2):
                    for a in range(AT):
                        nc.tensor.matmul(ps_lg[:, hh * 512:(hh + 1) * 512],
                                         wpc[:, a:a + 1], ha[:, a, hh * 512:(hh + 1) * 512],
                                         start=(a == 0), stop=(a == AT - 1))
                # -- softmax over n (free order (n,b)) --
                l_sb = wk.tile([1, N * BB], f32, tag="smx")
                nc.vector.tensor_add(l_sb[:], ps_lg[:], rmask[:])
                l3 = l_sb[:].rearrange("p (b2 n bp) -> p b2 n bp", b2=2, n=N)
                l3r = l3.transpose([0, 1, 3, 2])    # (b2, bp, n) - reduce innermost
                rmax = wk.tile([1, BB], f32, tag="smx2")   # (b2, bp)
                nc.vector.tensor_reduce(rmax[:].rearrange("p (b2 bp) -> p b2 bp", b2=2),
                                        l3r, mybir.AxisListType.X, mybir.AluOpType.max)
                rmx_b = rmax[:].rearrange("p (b2 bp) -> p b2 bp", b2=2).unsqueeze(
                    2).broadcast_to([1, 2, N, KH])
                ls2 = wk.tile([1, N * BB], f32, tag="smx")
                nc.vector.tensor_sub(
                    ls2[:].rearrange("p (b2 n bp) -> p b2 n bp", b2=2, n=N), l3, rmx_b)
                e_sb = wk.tile([1, N * BB], f32, tag="smx")
                nc.scalar.activation(e_sb[:], ls2[:], AF.Exp)
                ssum = wk.tile([1, BB], f32, tag="smx2")
                nc.vector.tensor_reduce(
                    ssum[:].rearrange("p (b2 bp) -> p b2 bp", b2=2),
                    e_sb[:].rearrange("p (b2 n bp) -> p b2 n bp", b2=2, n=N).transpose(
                        [0, 1, 3, 2]),
                    mybir.AxisListType.X, mybir.AluOpType.add)
                rinv = wk.tile([1, BB], f32, tag="smx2")
                nc.vector.reciprocal(rinv[:], ssum[:])
                att = wk.tile([1, N * BB], bf16, tag="smx")
                nc.vector.tensor_mul(
                    att[:].rearrange("p (b2 n bp) -> p b2 n bp", b2=2, n=N),
                    e_sb[:].rearrange("p (b2 n bp) -> p b2 n bp", b2=2, n=N),
                    rinv[:].rearrange("p (b2 bp) -> p b2 bp", b2=2).unsqueeze(
                        2).broadcast_to([1, 2, N, KH]))
                # -- att to block-diag [128, 16] via 2 SBUF->SBUF dmas --
                for b2 in range(2):
                    nc.sync.dma_start(
                        attT2z[b2 * N:(b2 + 1) * N, b2::2],
                        att[0:1, b2 * 512:(b2 + 1) * 512].rearrange(
                            "o (n bp) -> o n bp", n=N))
                # -- ctx: block-diag pairs --
                ps_cx = pssm.tile([P, KV * BB], f32, tag="cx")
                for bp in range(KH):
                    for vc in range(KV):
                        nc.tensor.matmul(
                            ps_cx[:, vc * BB + 2 * bp:vc * BB + 2 * bp + 2],
                            visn[:, bp, vc * P:(vc + 1) * P],
                            attT2z[:, 2 * bp:2 * bp + 2], start=True, stop=True)
                ctxT = wk.tile([P, KV * BB], bf16, tag="ctxT")
                nc.vector.tensor_copy(ctxT[:], ps_cx[:])
                # -- gate preacts: ps + pa accumulated per j-tile --
                ps_pre = pspre.tile([P, JT, BB], f32, tag="pre")
                for j in range(JT):
                    wat = wap.tile([P, KV, P], bf16, tag="wa")
                    nc.sync.dma_start(wat[:], dt_["waT"][j])
                    for k in range(KH):
                        nc.tensor.matmul(ps_pre[:, j, :], wsT[:, j, k, :],
                                         hTb[cur][:, k, :],
                                         start=(k == 0), stop=False)
                    for vc in range(KV):
                        nc.tensor.matmul(ps_pre[:, j, :], wat[:, vc, :],
                                         ctxT[:, vc * BB:(vc + 1) * BB],
                                         start=False, stop=(vc == KV - 1))
                # -- pre = psum + piT(t) --
                pit = wk.tile([P, JI, BB], bf16, tag="pit_t")
                nc.sync.dma_start(pit[:], piT_dram[t])
                pre = wk.tile([P, JT, BB], f32, tag="pre_sb")
                nc.vector.tensor_add(pre[:], ps_pre[:], pit[:, 0:JT, :])
                # -- gates (order i,f,o,hw,m) --
                gs = wk.tile([P, 4 * KH, BB], f32, tag="gs")
                nc.scalar.activation(gs[:], pre[:, 0:4 * KH, :], AF.Sigmoid)
                gm = wk.tile([P, KH, BB], f32, tag="gm")
                nc.scalar.activation(gm[:], pre[:, 4 * KH:5 * KH, :], AF.Tanh)
                t1 = wk.tile([P, KH, BB], f32, tag="gtmp", bufs=6, name="t1")
                nc.vector.tensor_mul(t1[:], gs[:, 0:KH, :], gm[:])        # i*m
                t2 = wk.tile([P, KH, BB], f32, tag="gtmp", bufs=6, name="t2")
                nc.vector.tensor_mul(t2[:], gs[:, KH:2 * KH, :], cT[cur][:])  # f*c
                cn = wk.tile([P, KH, BB], f32, tag="cn")
                nc.vector.tensor_add(cn[:], t1[:], t2[:])                 # mem
                tm = wk.tile([P, KH, BB], f32, tag="gtmp", bufs=6, name="tm")
                nc.scalar.activation(tm[:], cn[:], AF.Tanh)
                op_ = wk.tile([P, KH, BB], f32, tag="gtmp", bufs=6, name="op")
                nc.vector.tensor_mul(op_[:], gs[:, 2 * KH:3 * KH, :], tm[:])  # o*tanh
                pi6 = pit[:, JT:JI, :]
                d1 = wk.tile([P, KH, BB], f32, tag="gtmp", bufs=6, name="d1")
                nc.vector.tensor_sub(d1[:], op_[:], pi6)
                d2 = wk.tile([P, KH, BB], f32, tag="gtmp", bufs=6, name="d2")
                nc.vector.tensor_mul(d2[:], gs[:, 3 * KH:4 * KH, :], d1[:])
                oh = wk.tile([P, KH, BB], f32, tag="oh")
                nc.vector.tensor_add(oh[:], d2[:], pi6)                   # highway out
                # -- mask & state update --
                vm = vmask[:, t * BB:(t + 1) * BB].unsqueeze(1).broadcast_to([P, KH, BB])
                m1 = wk.tile([P, KH, BB], f32, tag="gtmp", bufs=6, name="m1")
                nc.vector.tensor_sub(m1[:], oh[:], hT[cur][:])
                m2 = wk.tile([P, KH, BB], f32, tag="gtmp", bufs=6, name="m2")
                nc.vector.tensor_mul(m2[:], vm, m1[:])
                nc.vector.tensor_add(hT[nxt][:], hT[cur][:], m2[:])
                m3 = wk.tile([P, KH, BB], f32, tag="gtmp", bufs=6, name="m3")
                nc.vector.tensor_sub(m3[:], cn[:], cT[cur][:])
                m4 = wk.tile([P, KH, BB], f32, tag="gtmp", bufs=6, name="m4")
                nc.vector.tensor_mul(m4[:], vm, m3[:])
                nc.vector.tensor_add(cT[nxt][:], cT[cur][:], m4[:])
                nc.vector.tensor_copy(hTb[nxt][:], hT[nxt][:])
                # -- y_t --
                ps_y = pslg.tile([1, BB], f32, tag="y")
                for k in range(KH):
                    nc.tensor.matmul(ps_y[:], woc[:, k:k + 1], hTb[nxt][:, k, :],
                                     start=(k == 0), stop=(k == KH - 1))
                yt = wk.tile([1, BB], f32, tag="smx2")
                nc.vector.tensor_scalar_add(yt[:], ps_y[:], float(bo_val))
                nc.vector.tensor_mul(y_sb[:, t * BB:(t + 1) * BB], yt[:],
                                     vmask[0:1, t * BB:(t + 1) * BB])

            nc.sync.dma_start(y_dram[:], y_sb[:])

    if do_compile:
        nc.compile()
    return nc


def run(inputs, trace=False):
    bo_val = float(np.asarray(inputs["bo"]).reshape(-1)[0])
    key = ("v1", bo_val)
    if key not in _CACHE:
        _CACHE[key] = _build(bo_val)
    nc = _CACHE[key]

    in_maps = [_prep_core_inputs(inputs, c) for c in range(NC_)]
    from concourse import bass_utils
    res = bass_utils.run_bass_kernel_spmd(
        nc, in_maps, core_ids=list(range(NC_)), trace=trace)
    y = np.empty((B, T, 1), np.float32)
    for c in range(NC_):
        yc = np.asarray(res.results[c]["y"], np.float32).reshape(T, BB)  # (t, b)
        y[c * BB:(c + 1) * BB, :, 0] = yc.T
    return y, res


def kernel(**inputs):
    return run(inputs, trace=False)[0]


# revision 19
# speedup vs baseline: 1.3168x; 1.3168x over previous
"""AttentionRNN Trainium2 kernel: 8-core data-parallel over batch.

Per-core layout (b=16 samples/core), everything feature-on-partition:
  - recurrent state hT/cT: [128, 8, 16]  (h-chunk on partitions, (k,b) free)
  - gate preacts psum:      [128, 40, 16] (5 gates x 8 h-chunks on j-tiles)
  - weights pre-transposed/gate-permuted on host into lhsT tile layouts (bf16)
  - pi = x@Wi.T + all biases precomputed once into DRAM scratch, bf16
  - attention: va (a-on-partition), logits via PE dot with Wp, softmax on
    one partition, att transposed to [64n,16b] via tiny SBUF->SBUF DMA,
    ctx via per-(b,vchunk) matmuls with visual as stationary operand.
Gate order permuted to (i, f, o, hw, m) so sigmoid is one contiguous slab.
"""

import os
import sys
import numpy as np

sys.path.insert(0, "/opt/trn_rl_repo")

B, T, D_IN, H, V, A, N = 128, 32, 1024, 1024, 2048, 512, 64
NC_ = 8
BB = B // NC_          # 16 samples per core
P = 128
KH = H // P            # 8  h-chunks
KV = V // P            # 16 v-chunks
AT = A // P            # 4  a-tiles
JT = 5 * H // P        # 40 gate j-tiles
JI = 6 * H // P        # 48 pi j-tiles (40 gates + 8 highway)
NTB = T * BB           # 512 (t,b) columns

# gate permutation: orig rows blocks (i,f,m,o,hw) -> new order (i,f,o,hw,m)
GPERM = [0, 1, 3, 4, 2]


def _orig_row_base(jt):
    """DRAM row base in the original 5H (or 6H) weight for new j-tile jt."""
    if jt < JT:
        g_new, hc = jt // KH, jt % KH
        return GPERM[g_new] * H + hc * P
    # highway block (pi only): orig rows 5H..6H
    return 5 * H + (jt - JT) * P


def _prep_core_inputs(inputs, core):
    f32, bf16 = np.float32, np.bfloat16 if hasattr(np, "bfloat16") else None
    import ml_dtypes
    bf16 = ml_dtypes.bfloat16

    sl = slice(core * BB, (core + 1) * BB)
    x = np.asarray(inputs["x"], f32)[sl]          # [16,32,1024]
    vis = np.asarray(inputs["visual"], f32)[sl]   # [16,64,2048]
    ls = np.asarray(inputs["ls_rois"])[sl].astype(np.int64)
    sq = np.asarray(inputs["seq_lens"])[sl].astype(np.int64)

    Wi = np.asarray(inputs["Wi"], f32); bi = np.asarray(inputs["bi"], f32)
    Ws = np.asarray(inputs["Ws"], f32); bs = np.asarray(inputs["bs"], f32)
    Wa = np.asarray(inputs["Wa"], f32); ba = np.asarray(inputs["ba"], f32)
    Wv = np.asarray(inputs["Wv"], f32); bv = np.asarray(inputs["bv"], f32)
    Wh = np.asarray(inputs["Wh"], f32); bh = np.asarray(inputs["bh"], f32)
    Wp = np.asarray(inputs["Wp"], f32)
    W0h = np.asarray(inputs["W0h"], f32); b0h = np.asarray(inputs["b0h"], f32)
    W0c = np.asarray(inputs["W0c"], f32); b0c = np.asarray(inputs["b0c"], f32)
    Wo = np.asarray(inputs["Wo"], f32)

    m = {}
    # xT [128, 8, 512]: xT[p,k,t*16+b] = x[b,t,k*128+p]
    xt = x.transpose(2, 1, 0).reshape(KH, P, T * BB)        # [k*p? ] careful
    # x.transpose(2,1,0): [1024 d, 32 t, 16 b] -> reshape [8, 128, 32*16]
    xt = x.transpose(2, 1, 0).reshape(KH, P, T, BB).reshape(KH, P, NTB)
    m["xT"] = np.ascontiguousarray(xt.transpose(1, 0, 2)).astype(bf16)  # [128,8,512]

    # wiT [48, 128, 8, 128]: wiT[j,p,k,q] = Wi[orig_row(j)+q, k*128+p]
    wiT = np.empty((JI, P, KH, P), f32)
    for j in range(JI):
        r0 = _orig_row_base(j)
        blk = Wi[r0:r0 + P, :].reshape(P, KH, P)            # [q, k, p]
        wiT[j] = blk.transpose(2, 1, 0)                     # [p, k, q]
    m["wiT"] = wiT.astype(bf16)

    # wsT [128, 40, 8, 128]: wsT[p,j,k,q] = Ws[orig_row(j)+q, k*128+p]
    wsT = np.empty((JT, P, KH, P), f32)
    for j in range(JT):
        r0 = _orig_row_base(j)
        wsT[j] = Ws[r0:r0 + P, :].reshape(P, KH, P).transpose(2, 1, 0)
    m["wsT"] = np.ascontiguousarray(wsT.transpose(1, 0, 2, 3)).astype(bf16)  # [128,40,8,128]

    # waT [40, 128, 16, 128]: waT[j,p,v,q] = Wa[orig_row(j)+q, v*128+p]
    waT = np.empty((JT, P, KV, P), f32)
    for j in range(JT):
        r0 = _orig_row_base(j)
        waT[j] = Wa[r0:r0 + P, :].reshape(P, KV, P).transpose(2, 1, 0)
    m["waT"] = waT.astype(bf16)

    # whT [128, 4, 8, 128]: whT[p,a,k,q] = Wh[a*128+q, k*128+p]
    whT = Wh.reshape(AT, P, KH, P).transpose(3, 0, 2, 1)
    m["whT"] = np.ascontiguousarray(whT).astype(bf16)

    # w0T [16, 128, 16, 128]: j2<8 -> W0h, j2>=8 -> W0c; w0T[j2,p,k,q]=W[j2%8*128+q, k*128+p]
    w0 = np.concatenate([W0h.reshape(KH, P, KV, P), W0c.reshape(KH, P, KV, P)], 0)
    m["w0T"] = np.ascontiguousarray(w0.transpose(0, 3, 2, 1)).astype(bf16)

    # wvT [4, 128, 16, 128]: wvT[a,p,v,q] = Wv[a*128+q, v*128+p]
    m["wvT"] = np.ascontiguousarray(
        Wv.reshape(AT, P, KV, P).transpose(0, 3, 2, 1)).astype(bf16)

    # visn [128, 8, 2048] bf16 packed pairs: visn[b2*64+n, bp, v] = visual[2*bp+b2, n, v]
    vp = vis.reshape(8, 2, N, V).transpose(1, 2, 0, 3)       # [b2, n, bp, v]
    m["visn"] = np.ascontiguousarray(vp.reshape(P, 8, V)).astype(bf16)

    # visT [128, 16, 16, 64]: visT[p,vc,b,n] = visual[b,n,vc*128+p]
    m["visT"] = np.ascontiguousarray(
        vis.reshape(BB, N, KV, P).transpose(3, 2, 0, 1)).astype(bf16)

    # wpc [128, 4], woc [128, 8]
    m["wpc"] = np.ascontiguousarray(Wp[0].reshape(AT, P).T).astype(bf16)
    m["woc"] = np.ascontiguousarray(Wo[0].reshape(KH, P).T).astype(bf16)

    # wroisT [128, 16] bf16 block-diag: col 2*bp+b2, row b2*64+n
    wr = (np.arange(N)[:, None] < ls[None, :]) / ls[None, :].astype(f32)  # [n, b]
    wz = np.zeros((2, N, BB), f32)
    for b in range(BB):
        wz[b % 2, :, b] = wr[:, b]
    m["wroisT"] = wz.reshape(P, BB).astype(bf16)

    # btot [128, 48] f32: folded bias for pi phase (bi + bs + ba on gate tiles)
    btot = np.empty((P, JI), f32)
    for j in range(JI):
        r0 = _orig_row_base(j)
        b = bi[r0:r0 + P].copy()
        if j < JT:
            b += bs[r0:r0 + P] + ba[r0:r0 + P]
        btot[:, j] = b
    m["btot"] = btot

    # bvh [128, 4]: bv + bh per a-tile
    m["bvh"] = np.ascontiguousarray((bv + bh).reshape(AT, P).T, f32)

    # b0 [128, 16]
    m["b0"] = np.ascontiguousarray(
        np.concatenate([b0h.reshape(KH, P), b0c.reshape(KH, P)], 0).T, f32)

    # rmask [1, 1024] f32, free order (b2, n, bp): idx = b2*512 + n*8 + bp
    rm = np.where(np.arange(N)[:, None] < ls[None, :], 0.0, -1e9).astype(f32)  # [n,b]
    rm2 = np.empty((2, N, 8), f32)
    for b in range(BB):
        rm2[b % 2, :, b // 2] = rm[:, b]
    m["rmask"] = rm2.reshape(1, N * BB)

    # vmask [128, 512] bf16 over (t,b), replicated on partitions
    vm = (np.arange(T)[:, None] < sq[None, :]).astype(f32)   # [32,16]
    m["vmask"] = np.broadcast_to(vm.reshape(1, NTB), (P, NTB)).astype(bf16)

    return m


_CACHE = {}


def _build(bo_val, do_compile=True):
    from contextlib import ExitStack
    import concourse.bacc as bacc
    import concourse.tile as tile
    from concourse import mybir

    nc = bacc.Bacc("TRN2", target_bir_lowering=False, debug=False)
    bf16, f32 = mybir.dt.bfloat16, mybir.dt.float32
    AF = mybir.ActivationFunctionType

    dt_ = {}
    def din(name, shape, dt=bf16):
        dt_[name] = nc.dram_tensor(name, shape, dt, kind="ExternalInput").ap()

    din("xT", [P, KH, NTB]); din("wiT", [JI, P, KH, P])
    din("wsT", [P, JT, KH, P]); din("waT", [JT, P, KV, P])
    din("whT", [P, AT, KH, P]); din("w0T", [2 * KH, P, KV, P])
    din("wvT", [AT, P, KV, P]); din("visn", [P, KH, V])
    din("visT", [P, KV, BB, N])
    din("wpc", [P, AT]); din("woc", [P, KH]); din("wroisT", [P, BB])
    din("btot", [P, JI], f32); din("bvh", [P, AT], f32); din("b0", [P, 2 * KH], f32)
    din("rmask", [1, N * BB], f32); din("vmask", [P, NTB])

    y_dram = nc.dram_tensor("y", [1, NTB], f32, kind="ExternalOutput").ap()

    with ExitStack() as stk:
        tc = stk.enter_context(tile.TileContext(nc))
        const = stk.enter_context(tc.tile_pool(name="const", bufs=1))
        dramp = stk.enter_context(tc.tile_pool(name="dscr", bufs=1, space="DRAM"))
        psum = stk.enter_context(tc.tile_pool(name="psum", bufs=2, space="PSUM"))
        piT_dram = dramp.tile([T, P, JI, BB], bf16)
        va2_dram = dramp.tile([JT, P, KH, P], bf16)

        # ---- resident consts ----
        wsT = const.tile([P, JT, KH, P], bf16)
        nc.sync.dma_start(wsT[:], dt_["wsT"][:])
        whT = const.tile([P, AT, KH, P], bf16)
        nc.sync.dma_start(whT[:], dt_["whT"][:])
        visn = const.tile([P, KH, V], bf16)
        nc.sync.dma_start(visn[:], dt_["visn"][:])
        wpc = const.tile([P, AT], bf16)
        nc.sync.dma_start(wpc[:], dt_["wpc"][:])
        woc = const.tile([P, KH], bf16)
        nc.sync.dma_start(woc[:], dt_["woc"][:])
        wroisT = const.tile([P, BB], bf16)
        nc.sync.dma_start(wroisT[:], dt_["wroisT"][:])
        btot = const.tile([P, JI], f32)
        nc.sync.dma_start(btot[:], dt_["btot"][:])
        bvh = const.tile([P, AT], f32)
        nc.sync.dma_start(bvh[:], dt_["bvh"][:])
        b0 = const.tile([P, 2 * KH], f32)
        nc.sync.dma_start(b0[:], dt_["b0"][:])
        rmask = const.tile([1, N * BB], f32)
        nc.sync.dma_start(rmask[:], dt_["rmask"][:])
        vmask = const.tile([P, NTB], bf16)
        nc.sync.dma_start(vmask[:], dt_["vmask"][:])
        attT2z = const.tile([P, BB], bf16)
        nc.vector.memset(attT2z[:], 0.0)
        va_sb = const.tile([P, AT, N * BB], bf16)   # relu input, a-on-partition
        y_sb = const.tile([1, NTB], f32)

        # state (ping-pong)
        hT = [const.tile([P, KH, BB], f32, tag=f"hT{i}", name=f"hT{i}") for i in range(2)]
        cT = [const.tile([P, KH, BB], f32, tag=f"cT{i}", name=f"cT{i}") for i in range(2)]
        hTb = [const.tile([P, KH, BB], bf16, tag=f"hTb{i}", name=f"hTb{i}") for i in range(2)]

        # ================= precompute =================
        with tc.tile_pool(name="pre_sbuf", bufs=3) as pp, \
             tc.tile_pool(name="pre_big", bufs=1) as pb:

            # ---- mean visual (as matmul with wrois), into mvT bf16 [128, 16v,16b] ----
            ps_mv = psum.tile([P, KV * BB], f32, tag="small")
            for bp in range(KH):
                for vc in range(KV):
                    nc.tensor.matmul(
                        ps_mv[:, vc * BB + 2 * bp:vc * BB + 2 * bp + 2],
                        visn[:, bp, vc * P:(vc + 1) * P],
                        wroisT[:, 2 * bp:2 * bp + 2], start=True, stop=True)
            mvT = pb.tile([P, KV * BB], bf16)
            nc.vector.tensor_copy(mvT[:], ps_mv[:])

            # ---- h0 / c0 ----
            for half in range(2):
                ps_h0 = psum.tile([P, KH, BB], f32, tag="small")
                for j in range(KH):
                    j2 = half * KH + j
                    w0t = pp.tile([P, KV, P], bf16, tag="w0s")
                    nc.sync.dma_start(w0t[:], dt_["w0T"][j2])
                    for k in range(KV):
                        nc.tensor.matmul(
                            ps_h0[:, j, :], w0t[:, k, :],
                            mvT[:, k * BB:(k + 1) * BB],
                            start=(k == 0), stop=(k == KV - 1))
                dst = hT[0] if half == 0 else cT[0]
                for j in range(KH):
                    j2 = half * KH + j
                    nc.scalar.activation(dst[:, j, :], ps_h0[:, j, :],
                                         AF.Identity, bias=b0[:, j2:j2 + 1])
            nc.vector.tensor_copy(hTb[0][:], hT[0][:])

            # ---- va = visual @ Wv.T + (bv+bh), free order (b2, n, bp) ----
            for a in range(AT):
                wvt = pp.tile([P, KV, P], bf16, tag="wvs")
                nc.sync.dma_start(wvt[:], dt_["wvT"][a])
                ps_va = psum.tile([P, BB, N], f32, tag="big")
                for vc in range(KV):
                    vst = pp.tile([P, BB, N], bf16, tag="vst")
                    nc.sync.dma_start(vst[:], dt_["visT"][:, vc])
                    for hh in range(2):
                        nc.tensor.matmul(
                            ps_va[:, hh * KH:(hh + 1) * KH, :], wvt[:, vc, :],
                            vst[:, hh * KH:(hh + 1) * KH, :],
                            start=(vc == 0), stop=(vc == KV - 1))
                # write va col = b2*512 + n*8 + bp from psum (b, n); 2 ACTs by parity
                for b2 in range(2):
                    in_ap = ps_va[:, b2::2, :]
                    out_ap = va_sb[:, a, b2 * 512:(b2 + 1) * 512].rearrange(
                        "p (n bp) -> p bp n", n=N)
                    nc.scalar.activation(out_ap, in_ap, AF.Identity,
                                         bias=bvh[:, a:a + 1])

            # ---- VA2[b] = visual[b] @ Wa.T -> transposed tiles in DRAM ----
            # va2_dram [40 jt, 128 (b2*64+n), 8 bp, 128 jq]
            for jt in range(JT):
                wat = pp.tile([P, KV, P], bf16, tag="wvs", name="wat_pre")
                nc.sync.dma_start(wat[:], dt_["waT"][jt])
                ps_va2 = psum.tile([P, BB, N], f32, tag="big", name="ps_va2")
                for vc in range(KV):
                    vst = pp.tile([P, BB, N], bf16, tag="vst", name="vst2")
                    nc.sync.dma_start(vst[:], dt_["visT"][:, vc])
                    for hh in range(2):
                        nc.tensor.matmul(
                            ps_va2[:, hh * KH:(hh + 1) * KH, :], wat[:, vc, :],
                            vst[:, hh * KH:(hh + 1) * KH, :],
                            start=(vc == 0), stop=(vc == KV - 1))
                va2sb = pp.tile([P, BB, N], bf16, tag="va2sb")
                nc.vector.tensor_copy(va2sb[:], ps_va2[:])
                va2t = pp.tile([P, KH, P], bf16, tag="va2t")
                for bp in range(KH):
                    nc.sync.dma_start_transpose(
                        va2t[:, bp, :],
                        va2sb[:, 2 * bp:2 * bp + 2, :].rearrange("p a b -> p (a b)"))
                nc.sync.dma_start(va2_dram[jt], va2t[:])

            # ---- pi precompute into DRAM scratch ----
            xT_s = pb.tile([P, KH, NTB], bf16)
            nc.sync.dma_start(xT_s[:], dt_["xT"][:])
            for j in range(JI):
                wit = pp.tile([P, KH, P], bf16, tag="wis")
                nc.sync.dma_start(wit[:], dt_["wiT"][j])
                ps_pi = psum.tile([P, NTB], f32, tag="big")
                for k in range(KH):
                    nc.tensor.matmul(ps_pi[:], wit[:, k, :], xT_s[:, k, :],
                                     start=(k == 0), stop=(k == KH - 1))
                pit = pp.tile([P, NTB], bf16, tag="pit")
                nc.scalar.activation(pit[:], ps_pi[:], AF.Identity,
                                     bias=btot[:, j:j + 1])
                # scatter to [T, P, j, BB]
                dst = piT_dram[:, :, j, :].transpose([1, 0, 2])  # iter (p, t, b)
                nc.sync.dma_start(dst, pit[:].rearrange("p (t b) -> p t b", t=T))

        # ================= scan =================
        tc.strict_bb_all_engine_barrier()
        with tc.tile_pool(name="wa_pool", bufs=3) as wap, \
             tc.tile_pool(name="work", bufs=2) as wk:

            ps_he = psum.tile([P, AT * BB], f32, tag="small")
            ps_pre = psum.tile([P, JT, BB], f32, tag="big")
            ps_a = psum.tile([P, JT, BB], f32, tag="big")
            ps_lg = psum.tile([1, N * BB], f32, tag="lg", bufs=1)
            ps_y = psum.tile([1, BB], f32, tag="small")
            for t in range(T):
                cur, nxt = t % 2, (t + 1) % 2
                # -- he --
                for a in range(AT):
                    for k in range(KH):
                        nc.tensor.matmul(ps_he[:, a * BB:(a + 1) * BB],
                                         whT[:, a, k, :], hTb[cur][:, k, :],
                                         start=(k == 0), stop=(k == KH - 1))
                # -- ps (Ws) gate preacts: first slice early (PE fill while ha on DVE) --
                for j in range(KH):
                    for k in range(KH):
                        nc.tensor.matmul(ps_pre[:, j, :], wsT[:, j, k, :],
                                         hTb[cur][:, k, :],
                                         start=(k == 0), stop=(k == KH - 1))
                # -- ha = relu(va + he), [128, (n,b)] per a; bf16 out --
                ha = wk.tile([P, AT, N * BB], bf16, tag="ha", bufs=1)
                for a in range(AT):
                    tmp = wk.tile([P, N * BB], f32, tag="hatmp")
                    he_b = ps_he[:, a * BB:(a + 1) * BB].rearrange(
                        "p (bp b2) -> p b2 bp", b2=2).unsqueeze(2).broadcast_to(
                        [P, 2, N, KH])
                    nc.vector.tensor_add(
                        tmp[:].rearrange("p (b2 n bp) -> p b2 n bp", b2=2, n=N),
                        va_sb[:, a, :].rearrange("p (b2 n bp) -> p b2 n bp", b2=2, n=N),
                        he_b)
                    nc.scalar.activation(ha[:, a, :], tmp[:], AF.Relu)
                # -- logits --
                for hh in range(2):
                    for a in range(AT):
                        nc.tensor.matmul(ps_lg[:, hh * 512:(hh + 1) * 512],
                                         wpc[:, a:a + 1], ha[:, a, hh * 512:(hh + 1) * 512],
                                         start=(a == 0), stop=(a == AT - 1))
                # -- remaining ps (Ws) MMs overlap with softmax/attT on DVE --
                for j in range(KH, JT):
                    for k in range(KH):
                        nc.tensor.matmul(ps_pre[:, j, :], wsT[:, j, k, :],
                                         hTb[cur][:, k, :],
                                         start=(k == 0), stop=(k == KH - 1))
                # -- softmax over n (free order (b2,n,bp)) --
                l_sb = wk.tile([1, N * BB], f32, tag="smx")
                nc.vector.tensor_add(l_sb[:], ps_lg[:], rmask[:])
                l3 = l_sb[:].rearrange("p (b2 n bp) -> p b2 n bp", b2=2, n=N)
                l3r = l3.transpose([0, 1, 3, 2])    # (b2, bp, n) - reduce innermost
                rmax = wk.tile([1, BB], f32, tag="smx2")   # (b2, bp)
                nc.vector.tensor_reduce(rmax[:].rearrange("p (b2 bp) -> p b2 bp", b2=2),
                                        l3r, mybir.AxisListType.X, mybir.AluOpType.max)
                rmx_b = rmax[:].rearrange("p (b2 bp) -> p b2 bp", b2=2).unsqueeze(
                    2).broadcast_to([1, 2, N, KH])
                ls2 = wk.tile([1, N * BB], f32, tag="smx")
                nc.vector.tensor_sub(
                    ls2[:].rearrange("p (b2 n bp) -> p b2 n bp", b2=2, n=N), l3, rmx_b)
                e_sb = wk.tile([1, N * BB], f32, tag="smx")
                nc.scalar.activation(e_sb[:], ls2[:], AF.Exp)
                ssum = wk.tile([1, BB], f32, tag="smx2")
                nc.vector.tensor_reduce(
                    ssum[:].rearrange("p (b2 bp) -> p b2 bp", b2=2),
                    e_sb[:].rearrange("p (b2 n bp) -> p b2 n bp", b2=2, n=N).transpose(
                        [0, 1, 3, 2]),
                    mybir.AxisListType.X, mybir.AluOpType.add)
                rinv = wk.tile([1, BB], f32, tag="smx2")
                nc.vector.reciprocal(rinv[:], ssum[:])
                att = wk.tile([1, N * BB], bf16, tag="smx")
                nc.vector.tensor_mul(
                    att[:].rearrange("p (b2 n bp) -> p b2 n bp", b2=2, n=N),
                    e_sb[:].rearrange("p (b2 n bp) -> p b2 n bp", b2=2, n=N),
                    rinv[:].rearrange("p (b2 bp) -> p b2 bp", b2=2).unsqueeze(
                        2).broadcast_to([1, 2, N, KH]))
                # -- att to block-diag [128, 16] via 2 SBUF->SBUF dmas --
                for b2 in range(2):
                    nc.sync.dma_start(
                        attT2z[b2 * N:(b2 + 1) * N, b2::2],
                        att[0:1, b2 * 512:(b2 + 1) * 512].rearrange(
                            "o (n bp) -> o n bp", n=N))
                # -- pa via precomputed VA2: block-diag pairs per j-tile --
                for j in range(JT):
                    va2t = wap.tile([P, KH, P], bf16, tag="wa")
                    nc.sync.dma_start(va2t[:], va2_dram[j])
                    for bp in range(KH):
                        nc.tensor.matmul(ps_a[:, j, 2 * bp:2 * bp + 2],
                                         va2t[:, bp, :],
                                         attT2z[:, 2 * bp:2 * bp + 2],
                                         start=True, stop=True)
                # -- pre = ps(Ws) + piT(t) + pa --
                pit = wk.tile([P, JI, BB], bf16, tag="pit_t")
                nc.sync.dma_start(pit[:], piT_dram[t])
                tpre = wk.tile([P, JT, BB], f32, tag="tpre")
                nc.vector.tensor_add(tpre[:], ps_pre[:], pit[:, 0:JT, :])
                pre = wk.tile([P, JT, BB], f32, tag="pre_sb")
                nc.vector.tensor_add(pre[:], tpre[:], ps_a[:])
                # -- gates (order i,f,o,hw,m) --
                gs = wk.tile([P, 4 * KH, BB], f32, tag="gs")
                nc.scalar.activation(gs[:], pre[:, 0:4 * KH, :], AF.Sigmoid)
                gm = wk.tile([P, KH, BB], f32, tag="gm")
                nc.scalar.activation(gm[:], pre[:, 4 * KH:5 * KH, :], AF.Tanh)
                t1 = wk.tile([P, KH, BB], f32, tag="gtmp", bufs=6, name="t1")
                nc.vector.tensor_mul(t1[:], gs[:, 0:KH, :], gm[:])        # i*m
                t2 = wk.tile([P, KH, BB], f32, tag="gtmp", bufs=6, name="t2")
                nc.vector.tensor_mul(t2[:], gs[:, KH:2 * KH, :], cT[cur][:])  # f*c
                cn = wk.tile([P, KH, BB], f32, tag="cn")
                nc.vector.tensor_add(cn[:], t1[:], t2[:])                 # mem
                tm = wk.tile([P, KH, BB], f32, tag="gtmp", bufs=6, name="tm")
                nc.scalar.activation(tm[:], cn[:], AF.Tanh)
                op_ = wk.tile([P, KH, BB], f32, tag="gtmp", bufs=6, name="op")
                nc.vector.tensor_mul(op_[:], gs[:, 2 * KH:3 * KH, :], tm[:])  # o*tanh
                pi6 = pit[:, JT:JI, :]
                d1 = wk.tile([P, KH, BB], f32, tag="gtmp", bufs=6, name="d1")
                nc.vector.tensor_sub(d1[:], op_[:], pi6)
                d2 = wk.tile([P, KH, BB], f32, tag="gtmp", bufs=6, name="d2")
                nc.vector.tensor_mul(d2[:], gs[:, 3 * KH:4 * KH, :], d1[:])
                oh = wk.tile([P, KH, BB], f32, tag="oh")
                nc.vector.tensor_add(oh[:], d2[:], pi6)                   # highway out
                # -- mask & state update --
                vm = vmask[:, t * BB:(t + 1) * BB].unsqueeze(1).broadcast_to([P, KH, BB])
                m1 = wk.tile([P, KH, BB], f32, tag="gtmp", bufs=6, name="m1")
                nc.vector.tensor_sub(m1[:], oh[:], hT[cur][:])
                m2 = wk.tile([P, KH, BB], f32, tag="gtmp", bufs=6, name="m2")
                nc.vector.tensor_mul(m2[:], vm, m1[:])
                nc.vector.tensor_add(hT[nxt][:], hT[cur][:], m2[:])
                m3 = wk.tile([P, KH, BB], f32, tag="gtmp", bufs=6, name="m3")
                nc.vector.tensor_sub(m3[:], cn[:], cT[cur][:])
                m4 = wk.tile([P, KH, BB], f32, tag="gtmp", bufs=6, name="m4")
                nc.vector.tensor_mul(m4[:], vm, m3[:])
                nc.vector.tensor_add(cT[nxt][:], cT[cur][:], m4[:])
                nc.vector.tensor_copy(hTb[nxt][:], hT[nxt][:])
                # -- y_t --
                for k in range(KH):
                    nc.tensor.matmul(ps_y[:], woc[:, k:k + 1], hTb[nxt][:, k, :],
                                     start=(k == 0), stop=(k == KH - 1))
                yt = wk.tile([1, BB], f32, tag="smx2")
                nc.vector.tensor_scalar_add(yt[:], ps_y[:], float(bo_val))
                nc.vector.tensor_mul(y_sb[:, t * BB:(t + 1) * BB], yt[:],
                                     vmask[0:1, t * BB:(t + 1) * BB])

            nc.sync.dma_start(y_dram[:], y_sb[:])

    if do_compile:
        nc.compile()
    return nc


def run(inputs, trace=False):
    bo_val = float(np.asarray(inputs["bo"]).reshape(-1)[0])
    key = ("v1", bo_val)
    if key not in _CACHE:
        _CACHE[key] = _build(bo_val)
    nc = _CACHE[key]

    in_maps = [_prep_core_inputs(inputs, c) for c in range(NC_)]
    from concourse import bass_utils
    res = bass_utils.run_bass_kernel_spmd(
        nc, in_maps, core_ids=list(range(NC_)), trace=trace)
    y = np.empty((B, T, 1), np.float32)
    for c in range(NC_):
        yc = np.asarray(res.results[c]["y"], np.float32).reshape(T, BB)  # (t, b)
        y[c * BB:(c + 1) * BB, :, 0] = yc.T
    return y, res


def kernel(**inputs):
    return run(inputs, trace=False)[0]


# revision 25
# speedup vs baseline: 1.3674x; 1.0385x over previous
"""AttentionRNN Trainium2 kernel: 8-core data-parallel over batch.

Per-core layout (b=16 samples/core), everything feature-on-partition:
  - recurrent state hT/cT: [128, 8, 16]  (h-chunk on partitions, (k,b) free)
  - gate preacts psum:      [128, 40, 16] (5 gates x 8 h-chunks on j-tiles)
  - weights pre-transposed/gate-permuted on host into lhsT tile layouts (bf16)
  - pi = x@Wi.T + all biases precomputed once into DRAM scratch, bf16
  - attention: va (a-on-partition), logits via PE dot with Wp, softmax on
    one partition, att transposed to [64n,16b] via tiny SBUF->SBUF DMA,
    ctx via per-(b,vchunk) matmuls with visual as stationary operand.
Gate order permuted to (i, f, o, hw, m) so sigmoid is one contiguous slab.
"""

import os
import sys
import numpy as np

sys.path.insert(0, "/opt/trn_rl_repo")

B, T, D_IN, H, V, A, N = 128, 32, 1024, 1024, 2048, 512, 64
NC_ = 8
BB = B // NC_          # 16 samples per core
P = 128
KH = H // P            # 8  h-chunks
KV = V // P            # 16 v-chunks
AT = A // P            # 4  a-tiles
JT = 5 * H // P        # 40 gate j-tiles
JI = 6 * H // P        # 48 pi j-tiles (40 gates + 8 highway)
NTB = T * BB           # 512 (t,b) columns

# gate permutation: orig rows blocks (i,f,m,o,hw) -> new order (i,f,o,hw,m)
GPERM = [0, 1, 3, 4, 2]


def _orig_row_base(jt):
    """DRAM row base in the original 5H (or 6H) weight for new j-tile jt."""
    if jt < JT:
        g_new, hc = jt // KH, jt % KH
        return GPERM[g_new] * H + hc * P
    # highway block (pi only): orig rows 5H..6H
    return 5 * H + (jt - JT) * P


def _prep_core_inputs(inputs, core):
    f32, bf16 = np.float32, np.bfloat16 if hasattr(np, "bfloat16") else None
    import ml_dtypes
    bf16 = ml_dtypes.bfloat16

    sl = slice(core * BB, (core + 1) * BB)
    x = np.asarray(inputs["x"], f32)[sl]          # [16,32,1024]
    vis = np.asarray(inputs["visual"], f32)[sl]   # [16,64,2048]
    ls = np.asarray(inputs["ls_rois"])[sl].astype(np.int64)
    sq = np.asarray(inputs["seq_lens"])[sl].astype(np.int64)

    Wi = np.asarray(inputs["Wi"], f32); bi = np.asarray(inputs["bi"], f32)
    Ws = np.asarray(inputs["Ws"], f32); bs = np.asarray(inputs["bs"], f32)
    Wa = np.asarray(inputs["Wa"], f32); ba = np.asarray(inputs["ba"], f32)
    Wv = np.asarray(inputs["Wv"], f32); bv = np.asarray(inputs["bv"], f32)
    Wh = np.asarray(inputs["Wh"], f32); bh = np.asarray(inputs["bh"], f32)
    Wp = np.asarray(inputs["Wp"], f32)
    W0h = np.asarray(inputs["W0h"], f32); b0h = np.asarray(inputs["b0h"], f32)
    W0c = np.asarray(inputs["W0c"], f32); b0c = np.asarray(inputs["b0c"], f32)
    Wo = np.asarray(inputs["Wo"], f32)

    m = {}
    # xT [128, 8, 512]: xT[p,k,t*16+b] = x[b,t,k*128+p]
    xt = x.transpose(2, 1, 0).reshape(KH, P, T * BB)        # [k*p? ] careful
    # x.transpose(2,1,0): [1024 d, 32 t, 16 b] -> reshape [8, 128, 32*16]
    xt = x.transpose(2, 1, 0).reshape(KH, P, T, BB).reshape(KH, P, NTB)
    m["xT"] = np.ascontiguousarray(xt.transpose(1, 0, 2)).astype(bf16)  # [128,8,512]

    # wiT [48, 128, 8, 128]: wiT[j,p,k,q] = Wi[orig_row(j)+q, k*128+p]
    wiT = np.empty((JI, P, KH, P), f32)
    for j in range(JI):
        r0 = _orig_row_base(j)
        blk = Wi[r0:r0 + P, :].reshape(P, KH, P)            # [q, k, p]
        wiT[j] = blk.transpose(2, 1, 0)                     # [p, k, q]
    m["wiT"] = wiT.astype(bf16)

    # wsT [128, 40, 8, 128]: wsT[p,j,k,q] = Ws[orig_row(j)+q, k*128+p]
    wsT = np.empty((JT, P, KH, P), f32)
    for j in range(JT):
        r0 = _orig_row_base(j)
        wsT[j] = Ws[r0:r0 + P, :].reshape(P, KH, P).transpose(2, 1, 0)
    m["wsT"] = np.ascontiguousarray(wsT.transpose(1, 0, 2, 3)).astype(bf16)  # [128,40,8,128]

    # waT [40, 128, 16, 128]: waT[j,p,v,q] = Wa[orig_row(j)+q, v*128+p]
    waT = np.empty((JT, P, KV, P), f32)
    for j in range(JT):
        r0 = _orig_row_base(j)
        waT[j] = Wa[r0:r0 + P, :].reshape(P, KV, P).transpose(2, 1, 0)
    m["waT"] = waT.astype(bf16)

    # whT [128, 4, 8, 128]: whT[p,a,k,q] = Wh[a*128+q, k*128+p]
    whT = Wh.reshape(AT, P, KH, P).transpose(3, 0, 2, 1)
    m["whT"] = np.ascontiguousarray(whT).astype(bf16)

    # w0T [16, 128, 16, 128]: j2<8 -> W0h, j2>=8 -> W0c; w0T[j2,p,k,q]=W[j2%8*128+q, k*128+p]
    w0 = np.concatenate([W0h.reshape(KH, P, KV, P), W0c.reshape(KH, P, KV, P)], 0)
    m["w0T"] = np.ascontiguousarray(w0.transpose(0, 3, 2, 1)).astype(bf16)

    # wvT [4, 128, 16, 128]: wvT[a,p,v,q] = Wv[a*128+q, v*128+p]
    m["wvT"] = np.ascontiguousarray(
        Wv.reshape(AT, P, KV, P).transpose(0, 3, 2, 1)).astype(bf16)

    # visn [128, 8, 2048] bf16 packed pairs: visn[b2*64+n, bp, v] = visual[2*bp+b2, n, v]
    vp = vis.reshape(8, 2, N, V).transpose(1, 2, 0, 3)       # [b2, n, bp, v]
    m["visn"] = np.ascontiguousarray(vp.reshape(P, 8, V)).astype(bf16)

    # visT [128, 16, 16, 64]: visT[p,vc,b,n] = visual[b,n,vc*128+p]
    m["visT"] = np.ascontiguousarray(
        vis.reshape(BB, N, KV, P).transpose(3, 2, 0, 1)).astype(bf16)

    # wpc [128, 4], woc [128, 8]
    m["wpc"] = np.ascontiguousarray(Wp[0].reshape(AT, P).T).astype(bf16)
    m["woc"] = np.ascontiguousarray(Wo[0].reshape(KH, P).T).astype(bf16)

    # wroisT [128, 16] bf16 block-diag: col 2*bp+b2, row b2*64+n
    wr = (np.arange(N)[:, None] < ls[None, :]) / ls[None, :].astype(f32)  # [n, b]
    wz = np.zeros((2, N, BB), f32)
    for b in range(BB):
        wz[b % 2, :, b] = wr[:, b]
    m["wroisT"] = wz.reshape(P, BB).astype(bf16)

    # btot [128, 48] f32: folded bias for pi phase (bi + bs + ba on gate tiles)
    btot = np.empty((P, JI), f32)
    for j in range(JI):
        r0 = _orig_row_base(j)
        b = bi[r0:r0 + P].copy()
        if j < JT:
            b += bs[r0:r0 + P] + ba[r0:r0 + P]
        btot[:, j] = b
    m["btot"] = btot

    # bvh [128, 4]: bv + bh per a-tile
    m["bvh"] = np.ascontiguousarray((bv + bh).reshape(AT, P).T, f32)

    # b0 [128, 16]
    m["b0"] = np.ascontiguousarray(
        np.concatenate([b0h.reshape(KH, P), b0c.reshape(KH, P)], 0).T, f32)

    # rmask [1, 1024] f32, free order (b2, n, bp): idx = b2*512 + n*8 + bp
    rm = np.where(np.arange(N)[:, None] < ls[None, :], 0.0, -1e9).astype(f32)  # [n,b]
    rm2 = np.empty((2, N, 8), f32)
    for b in range(BB):
        rm2[b % 2, :, b // 2] = rm[:, b]
    m["rmask"] = rm2.reshape(1, N * BB)

    # vmask [128, 512] bf16 over (t,b), replicated on partitions
    vm = (np.arange(T)[:, None] < sq[None, :]).astype(f32)   # [32,16]
    m["vmask"] = np.broadcast_to(vm.reshape(1, NTB), (P, NTB)).astype(bf16)

    return m


_CACHE = {}


def _build(bo_val, do_compile=True, t_steps=T):
    from contextlib import ExitStack
    import concourse.bacc as bacc
    import concourse.tile as tile
    from concourse import mybir

    nc = bacc.Bacc("TRN2", target_bir_lowering=False, debug=False)
    bf16, f32 = mybir.dt.bfloat16, mybir.dt.float32
    AF = mybir.ActivationFunctionType

    dt_ = {}
    def din(name, shape, dt=bf16):
        dt_[name] = nc.dram_tensor(name, shape, dt, kind="ExternalInput").ap()

    din("xT", [P, KH, NTB]); din("wiT", [JI, P, KH, P])
    din("wsT", [P, JT, KH, P]); din("waT", [JT, P, KV, P])
    din("whT", [P, AT, KH, P]); din("w0T", [2 * KH, P, KV, P])
    din("wvT", [AT, P, KV, P]); din("visn", [P, KH, V])
    din("visT", [P, KV, BB, N])
    din("wpc", [P, AT]); din("woc", [P, KH]); din("wroisT", [P, BB])
    din("btot", [P, JI], f32); din("bvh", [P, AT], f32); din("b0", [P, 2 * KH], f32)
    din("rmask", [1, N * BB], f32); din("vmask", [P, NTB])

    y_dram = nc.dram_tensor("y", [1, NTB], f32, kind="ExternalOutput").ap()

    with ExitStack() as stk:
        tc = stk.enter_context(tile.TileContext(nc))
        const = stk.enter_context(tc.tile_pool(name="const", bufs=1))
        dramp = stk.enter_context(tc.tile_pool(name="dscr", bufs=1, space="DRAM"))
        psum = stk.enter_context(tc.tile_pool(name="psum", bufs=2, space="PSUM"))
        piT_dram = dramp.tile([T, P, JI, BB], bf16)
        va2_dram = dramp.tile([JT, P, KH, P], bf16)

        # ---- resident consts ----
        wsT = const.tile([P, JT, KH, P], bf16)
        nc.sync.dma_start(wsT[:], dt_["wsT"][:])
        whT = const.tile([P, AT, KH, P], bf16)
        nc.sync.dma_start(whT[:], dt_["whT"][:])
        visn = const.tile([P, KH, V], bf16)
        nc.sync.dma_start(visn[:], dt_["visn"][:])
        wpc = const.tile([P, AT], bf16)
        nc.sync.dma_start(wpc[:], dt_["wpc"][:])
        woc = const.tile([P, KH], bf16)
        nc.sync.dma_start(woc[:], dt_["woc"][:])
        wroisT = const.tile([P, BB], bf16)
        nc.sync.dma_start(wroisT[:], dt_["wroisT"][:])
        btot = const.tile([P, JI], f32)
        nc.sync.dma_start(btot[:], dt_["btot"][:])
        bvh = const.tile([P, AT], f32)
        nc.sync.dma_start(bvh[:], dt_["bvh"][:])
        b0 = const.tile([P, 2 * KH], f32)
        nc.sync.dma_start(b0[:], dt_["b0"][:])
        rmask = const.tile([1, N * BB], f32)
        nc.sync.dma_start(rmask[:], dt_["rmask"][:])
        vmask = const.tile([P, NTB], bf16)
        nc.sync.dma_start(vmask[:], dt_["vmask"][:])
        attT2z = const.tile([P, BB], bf16)
        nc.vector.memset(attT2z[:], 0.0)
        va_sb = const.tile([P, AT, N * BB], bf16)   # relu input, a-on-partition
        y_sb = const.tile([1, NTB], f32)
        nc.vector.memset(y_sb[:], 0.0)

        # state (ping-pong)
        hT = [const.tile([P, KH, BB], f32, tag=f"hT{i}", name=f"hT{i}") for i in range(2)]
        cT = [const.tile([P, KH, BB], f32, tag=f"cT{i}", name=f"cT{i}") for i in range(2)]
        hTb = [const.tile([P, KH, BB], bf16, tag=f"hTb{i}", name=f"hTb{i}") for i in range(2)]

        # ================= precompute =================
        with tc.tile_pool(name="pre_sbuf", bufs=3) as pp, \
             tc.tile_pool(name="pre_big", bufs=1) as pb:

            # ---- mean visual (as matmul with wrois), into mvT bf16 [128, 16v,16b] ----
            ps_mv = psum.tile([P, KV * BB], f32, tag="small")
            for bp in range(KH):
                for vc in range(KV):
                    nc.tensor.matmul(
                        ps_mv[:, vc * BB + 2 * bp:vc * BB + 2 * bp + 2],
                        visn[:, bp, vc * P:(vc + 1) * P],
                        wroisT[:, 2 * bp:2 * bp + 2], start=True, stop=True)
            mvT = pb.tile([P, KV * BB], bf16)
            nc.vector.tensor_copy(mvT[:], ps_mv[:])

            # ---- h0 / c0 ----
            for half in range(2):
                ps_h0 = psum.tile([P, KH, BB], f32, tag="small")
                for j in range(KH):
                    j2 = half * KH + j
                    w0t = pp.tile([P, KV, P], bf16, tag="w0s")
                    nc.sync.dma_start(w0t[:], dt_["w0T"][j2])
                    for k in range(KV):
                        nc.tensor.matmul(
                            ps_h0[:, j, :], w0t[:, k, :],
                            mvT[:, k * BB:(k + 1) * BB],
                            start=(k == 0), stop=(k == KV - 1))
                dst = hT[0] if half == 0 else cT[0]
                for j in range(KH):
                    j2 = half * KH + j
                    nc.scalar.activation(dst[:, j, :], ps_h0[:, j, :],
                                         AF.Identity, bias=b0[:, j2:j2 + 1])
            nc.vector.tensor_copy(hTb[0][:], hT[0][:])

            # ---- va = visual @ Wv.T + (bv+bh), free order (b2, n, bp) ----
            for a in range(AT):
                wvt = pp.tile([P, KV, P], bf16, tag="wvs")
                nc.sync.dma_start(wvt[:], dt_["wvT"][a])
                ps_va = psum.tile([P, BB, N], f32, tag="big")
                for vc in range(KV):
                    vst = pp.tile([P, BB, N], bf16, tag="vst")
                    nc.sync.dma_start(vst[:], dt_["visT"][:, vc])
                    for hh in range(2):
                        nc.tensor.matmul(
                            ps_va[:, hh * KH:(hh + 1) * KH, :], wvt[:, vc, :],
                            vst[:, hh * KH:(hh + 1) * KH, :],
                            start=(vc == 0), stop=(vc == KV - 1))
                # write va col = b2*512 + n*8 + bp from psum (b, n); 2 ACTs by parity
                for b2 in range(2):
                    in_ap = ps_va[:, b2::2, :]
                    out_ap = va_sb[:, a, b2 * 512:(b2 + 1) * 512].rearrange(
                        "p (n bp) -> p bp n", n=N)
                    nc.scalar.activation(out_ap, in_ap, AF.Identity,
                                         bias=bvh[:, a:a + 1])

            # ---- VA2[b] = visual[b] @ Wa.T -> transposed tiles in DRAM ----
            # va2_dram [40 jt, 128 (b2*64+n), 8 bp, 128 jq]
            for jt in range(JT):
                wat = pp.tile([P, KV, P], bf16, tag="wvs", name="wat_pre")
                nc.sync.dma_start(wat[:], dt_["waT"][jt])
                ps_va2 = psum.tile([P, BB, N], f32, tag="big", name="ps_va2")
                for vc in range(KV):
                    vst = pp.tile([P, BB, N], bf16, tag="vst", name="vst2")
                    nc.sync.dma_start(vst[:], dt_["visT"][:, vc])
                    for hh in range(2):
                        nc.tensor.matmul(
                            ps_va2[:, hh * KH:(hh + 1) * KH, :], wat[:, vc, :],
                            vst[:, hh * KH:(hh + 1) * KH, :],
                            start=(vc == 0), stop=(vc == KV - 1))
                va2sb = pp.tile([P, BB, N], bf16, tag="va2sb")
                nc.vector.tensor_copy(va2sb[:], ps_va2[:])
                va2t = pp.tile([P, KH, P], bf16, tag="va2t")
                for bp in range(KH):
                    nc.sync.dma_start_transpose(
                        va2t[:, bp, :],
                        va2sb[:, 2 * bp:2 * bp + 2, :].rearrange("p a b -> p (a b)"))
                nc.sync.dma_start(va2_dram[jt], va2t[:])

            # ---- pi precompute into DRAM scratch ----
            xT_s = pb.tile([P, KH, NTB], bf16)
            nc.sync.dma_start(xT_s[:], dt_["xT"][:])
            for j in range(JI):
                wit = pp.tile([P, KH, P], bf16, tag="wis")
                nc.sync.dma_start(wit[:], dt_["wiT"][j])
                ps_pi = psum.tile([P, NTB], f32, tag="big")
                for k in range(KH):
                    nc.tensor.matmul(ps_pi[:], wit[:, k, :], xT_s[:, k, :],
                                     start=(k == 0), stop=(k == KH - 1))
                pit = pp.tile([P, NTB], bf16, tag="pit")
                nc.scalar.activation(pit[:], ps_pi[:], AF.Identity,
                                     bias=btot[:, j:j + 1])
                # scatter to [T, P, j, BB]
                dst = piT_dram[:, :, j, :].transpose([1, 0, 2])  # iter (p, t, b)
                nc.sync.dma_start(dst, pit[:].rearrange("p (t b) -> p t b", t=T))

        # ================= scan =================
        tc.strict_bb_all_engine_barrier()
        with tc.tile_pool(name="wa_pool", bufs=3) as wap, \
             tc.tile_pool(name="work", bufs=2) as wk:

            ps_he = psum.tile([P, AT * BB], f32, tag="small")
            ps_pre = psum.tile([P, JT, BB], f32, tag="big")
            ps_y = psum.tile([1, BB], f32, tag="small")
            ps_a = psum.tile([P, JT, BB], f32, tag="big")
            ps_lg = psum.tile([1, N * BB], f32, tag="lg", bufs=1)
            for t in range(t_steps):
                cur, nxt = t % 2, (t + 1) % 2
                # -- he --
                for a in range(AT):
                    for k in range(KH):
                        nc.tensor.matmul(ps_he[:, a * BB:(a + 1) * BB],
                                         whT[:, a, k, :], hTb[cur][:, k, :],
                                         start=(k == 0), stop=(k == KH - 1))
                # -- ps (Ws) gate preacts: first slice early (PE fill while ha on DVE) --
                for j in range(KH):
                    for k in range(KH):
                        nc.tensor.matmul(ps_pre[:, j, :], wsT[:, j, k, :],
                                         hTb[cur][:, k, :],
                                         start=(k == 0), stop=(k == KH - 1))
                # -- ha = relu(va + he): one add + one relu over all a --
                ha = wk.tile([P, AT, N * BB], bf16, tag="ha", bufs=1)
                tmp4 = wk.tile([P, AT, N * BB], bf16, tag="hatmp", bufs=1)
                for b2 in range(2):
                    he_b = ps_he[:].rearrange(
                        "p (a bp b2) -> p a bp b2", a=AT, b2=2)[:, :, :, b2].unsqueeze(
                        2).broadcast_to([P, AT, N, KH])
                    nc.vector.tensor_add(
                        tmp4[:, :, b2 * 512:(b2 + 1) * 512].rearrange(
                            "p q (n bp) -> p q n bp", n=N),
                        va_sb[:, :, b2 * 512:(b2 + 1) * 512].rearrange(
                            "p q (n bp) -> p q n bp", n=N),
                        he_b)
                nc.scalar.activation(ha[:], tmp4[:], AF.Relu)
                # -- logits --
                for hh in range(2):
                    for a in range(AT):
                        nc.tensor.matmul(ps_lg[:, hh * 512:(hh + 1) * 512],
                                         wpc[:, a:a + 1], ha[:, a, hh * 512:(hh + 1) * 512],
                                         start=(a == 0), stop=(a == AT - 1))
                # -- remaining ps (Ws) MMs overlap with softmax/attT on DVE --
                for j in range(KH, JT):
                    for k in range(KH):
                        nc.tensor.matmul(ps_pre[:, j, :], wsT[:, j, k, :],
                                         hTb[cur][:, k, :],
                                         start=(k == 0), stop=(k == KH - 1))
                # -- softmax over n (free order (b2,n,bp)) --
                l_sb = wk.tile([1, N * BB], f32, tag="smx")
                nc.vector.tensor_add(l_sb[:], ps_lg[:], rmask[:])
                l3 = l_sb[:].rearrange("p (b2 n bp) -> p b2 n bp", b2=2, n=N)
                l3r = l3.transpose([0, 1, 3, 2])    # (b2, bp, n) - reduce innermost
                rmax = wk.tile([1, BB], f32, tag="smx2")   # (b2, bp)
                nc.vector.tensor_reduce(rmax[:].rearrange("p (b2 bp) -> p b2 bp", b2=2),
                                        l3r, mybir.AxisListType.X, mybir.AluOpType.max)
                rmx_b = rmax[:].rearrange("p (b2 bp) -> p b2 bp", b2=2).unsqueeze(
                    2).broadcast_to([1, 2, N, KH])
                ls2 = wk.tile([1, N * BB], f32, tag="smx")
                nc.vector.tensor_sub(
                    ls2[:].rearrange("p (b2 n bp) -> p b2 n bp", b2=2, n=N), l3, rmx_b)
                e_sb = wk.tile([1, N * BB], f32, tag="smx")
                nc.scalar.activation(e_sb[:], ls2[:], AF.Exp)
                ssum = wk.tile([1, BB], f32, tag="smx2")
                nc.vector.tensor_reduce(
                    ssum[:].rearrange("p (b2 bp) -> p b2 bp", b2=2),
                    e_sb[:].rearrange("p (b2 n bp) -> p b2 n bp", b2=2, n=N).transpose(
                        [0, 1, 3, 2]),
                    mybir.AxisListType.X, mybir.AluOpType.add)
                rinv = wk.tile([1, BB], f32, tag="smx2")
                nc.vector.reciprocal(rinv[:], ssum[:])
                att = wk.tile([1, N * BB], bf16, tag="smx")
                nc.vector.tensor_mul(
                    att[:].rearrange("p (b2 n bp) -> p b2 n bp", b2=2, n=N),
                    e_sb[:].rearrange("p (b2 n bp) -> p b2 n bp", b2=2, n=N),
                    rinv[:].rearrange("p (b2 bp) -> p b2 bp", b2=2).unsqueeze(
                        2).broadcast_to([1, 2, N, KH]))
                # -- att to block-diag [128, 16] via 2 SBUF->SBUF dmas --
                for b2 in range(2):
                    nc.sync.dma_start(
                        attT2z[b2 * N:(b2 + 1) * N, b2::2],
                        att[0:1, b2 * 512:(b2 + 1) * 512].rearrange(
                            "o (n bp) -> o n bp", n=N))
                # -- tpre = ps(Ws) + piT(t) (overlaps with VA2 matmuls) --
                pit = wk.tile([P, JI, BB], bf16, tag="pit_t")
                nc.sync.dma_start(pit[:], piT_dram[t])
                tpre = wk.tile([P, JT, BB], f32, tag="tpre")
                nc.vector.tensor_add(tpre[:], ps_pre[:], pit[:, 0:JT, :])
                # -- pa via precomputed VA2: block-diag pairs per j-tile --
                for j in range(JT):
                    va2t = wap.tile([P, KH, P], bf16, tag="wa")
                    nc.sync.dma_start(va2t[:], va2_dram[j])
                    for bp in range(KH):
                        nc.tensor.matmul(ps_a[:, j, 2 * bp:2 * bp + 2],
                                         va2t[:, bp, :],
                                         attT2z[:, 2 * bp:2 * bp + 2],
                                         start=True, stop=True)
                pre = wk.tile([P, JT, BB], f32, tag="pre_sb")
                nc.vector.tensor_add(pre[:], tpre[:], ps_a[:])
                # -- gates (order i,f,o,hw,m) --
                gs = wk.tile([P, 4 * KH, BB], f32, tag="gs")
                nc.scalar.activation(gs[:], pre[:, 0:4 * KH, :], AF.Sigmoid)
                gm = wk.tile([P, KH, BB], f32, tag="gm")
                nc.scalar.activation(gm[:], pre[:, 4 * KH:5 * KH, :], AF.Tanh)
                t1 = wk.tile([P, KH, BB], f32, tag="gtmp", bufs=6, name="t1")
                nc.vector.tensor_mul(t1[:], gs[:, 0:KH, :], gm[:])        # i*m
                t2 = wk.tile([P, KH, BB], f32, tag="gtmp", bufs=6, name="t2")
                nc.vector.tensor_mul(t2[:], gs[:, KH:2 * KH, :], cT[cur][:])  # f*c
                cn = wk.tile([P, KH, BB], f32, tag="cn")
                nc.vector.tensor_add(cn[:], t1[:], t2[:])                 # mem
                tm = wk.tile([P, KH, BB], f32, tag="gtmp", bufs=6, name="tm")
                nc.scalar.activation(tm[:], cn[:], AF.Tanh)
                op_ = wk.tile([P, KH, BB], f32, tag="gtmp", bufs=6, name="op")
                nc.vector.tensor_mul(op_[:], gs[:, 2 * KH:3 * KH, :], tm[:])  # o*tanh
                pi6 = pit[:, JT:JI, :]
                d1 = wk.tile([P, KH, BB], f32, tag="gtmp", bufs=6, name="d1")
                nc.vector.tensor_sub(d1[:], op_[:], pi6)
                d2 = wk.tile([P, KH, BB], f32, tag="gtmp", bufs=6, name="d2")
                nc.vector.tensor_mul(d2[:], gs[:, 3 * KH:4 * KH, :], d1[:])
                oh = wk.tile([P, KH, BB], f32, tag="oh")
                nc.vector.tensor_add(oh[:], d2[:], pi6)                   # highway out
                # -- mask & state update --
                vm = vmask[:, t * BB:(t + 1) * BB].unsqueeze(1).broadcast_to([P, KH, BB])
                m1 = wk.tile([P, KH, BB], f32, tag="gtmp", bufs=6, name="m1")
                nc.vector.tensor_sub(m1[:], oh[:], hT[cur][:])
                m2 = wk.tile([P, KH, BB], f32, tag="gtmp", bufs=6, name="m2")
                nc.vector.tensor_mul(m2[:], vm, m1[:])
                nc.vector.tensor_add(hT[nxt][:], hT[cur][:], m2[:])
                m3 = wk.tile([P, KH, BB], f32, tag="gtmp", bufs=6, name="m3")
                nc.vector.tensor_sub(m3[:], cn[:], cT[cur][:])
                m4 = wk.tile([P, KH, BB], f32, tag="gtmp", bufs=6, name="m4")
                nc.vector.tensor_mul(m4[:], vm, m3[:])
                nc.vector.tensor_add(cT[nxt][:], cT[cur][:], m4[:])
                nc.vector.tensor_copy(hTb[nxt][:], hT[nxt][:])
                # -- y_t --
                for k in range(KH):
                    nc.tensor.matmul(ps_y[:], woc[:, k:k + 1], hTb[nxt][:, k, :],
                                     start=(k == 0), stop=(k == KH - 1))
                yt = wk.tile([1, BB], f32, tag="smx2")
                nc.vector.tensor_scalar_add(yt[:], ps_y[:], float(bo_val))
                nc.vector.tensor_mul(y_sb[:, t * BB:(t + 1) * BB], yt[:],
                                     vmask[0:1, t * BB:(t + 1) * BB])

            nc.sync.dma_start(y_dram[:], y_sb[:])

    if do_compile:
        nc.compile()
    return nc


def run(inputs, trace=False):
    bo_val = float(np.asarray(inputs["bo"]).reshape(-1)[0])
    key = ("v1", bo_val)
    if key not in _CACHE:
        _CACHE[key] = _build(bo_val)
    nc = _CACHE[key]

    in_maps = [_prep_core_inputs(inputs, c) for c in range(NC_)]
    from concourse import bass_utils
    res = bass_utils.run_bass_kernel_spmd(
        nc, in_maps, core_ids=list(range(NC_)), trace=trace)
    y = np.empty((B, T, 1), np.float32)
    for c in range(NC_):
        yc = np.asarray(res.results[c]["y"], np.float32).reshape(T, BB)  # (t, b)
        y[c * BB:(c + 1) * BB, :, 0] = yc.T
    return y, res


def kernel(**inputs):
    return run(inputs, trace=False)[0]
